# revision 11
# baseline (speedup 1.0000x reference)
"""DeChunkLayer Trainium2 kernel.

Computation (per batch row b):
    proj = x[b] @ W.T + b                                  # [C, D]
    nproj = LayerNorm(proj) (= (proj-mean)*rsqrt(var+eps)*gamma + beta)
    out[t] = nproj[idx[t]] for valid t, beta for padded t  # [L, D]

Key insight: LayerNorm commutes with the ragged expansion (each output row is a
copy of a chunk row, so its stats equal the chunk row's stats). So we normalize
the C=2048 chunk rows once and then expand, instead of normalizing L=8192 rows.

The ragged expansion is computed on the TensorEngine as a block-sparse one-hot
matmul: for each 128-token output block we accumulate over the (very few)
128-chunk tiles the block draws from:  out_block = sum_ct OH[k,ct].T @ nproj[ct].
The one-hot piece matrices are built on the host from chunk_lengths (tiny input)
and fed as bf16 data; piece->tile wiring is baked at trace time (the kernel is
JIT-specialized per input, SPMD-uniform across the 8 cores: the piece list is
the union over cores, per-core one-hot data is zero where a core doesn't use a
piece). Sharding: data-parallel over the batch dim, one NeuronCore per row;
W/bias/gamma/beta replicated.
"""

import sys

for _p in ("/opt/trn_rl_repo",):
    if _p not in sys.path:
        sys.path.insert(0, _p)

import numpy as np
import ml_dtypes

import concourse.bass as bass
import concourse.tile as tile
from concourse import bacc, mybir
from concourse.bass_utils import run_bass_kernel_spmd

B, C, D, L = 8, 2048, 512, 8192
P = 128           # partitions
CT = C // P       # 16 chunk tiles
NT = L // P       # 64 output-token blocks
KT = D // P       # 4 contraction tiles
EPS = 1e-5

F32 = mybir.dt.float32
BF16 = mybir.dt.bfloat16
BF = ml_dtypes.bfloat16


# ----------------------------------------------------------------- host logic

def _build_indices(chunk_lengths: np.ndarray) -> np.ndarray:
    """idx[b, t] = source chunk row for output token t; C means 'beta row'."""
    cum = np.cumsum(chunk_lengths.astype(np.int64), axis=1)
    pos = np.arange(L)
    idx = np.empty((B, L), np.int32)
    for b in range(B):
        i = np.searchsorted(cum[b], pos, side="right").astype(np.int32)
        i = np.minimum(i, C - 1)
        i[pos >= cum[b, -1]] = C  # padded positions -> beta row
        idx[b] = i
    return idx


def _build_pieces(idx: np.ndarray):
    """Per 128-token block, the union (over cores) of touched 128-chunk tiles."""
    blocks = idx.reshape(B, NT, P)
    piece_meta = []  # list of (block k, chunk-tile ct)
    for k in range(NT):
        tiles = sorted(set((blocks[:, k, :] // P).ravel().tolist()))
        for ct in tiles:
            piece_meta.append((k, ct))
    return piece_meta


def _build_onehot(idx: np.ndarray, piece_meta) -> np.ndarray:
    """oh[b, j, p, tt] = 1 iff idx[b, 128k+tt] == 128*ct + j for piece p=(k,ct).

    Layout is partition-first ([128, NP, 128]) so the whole thing loads with one
    line-rate DMA and piece p is the SBUF slice [:, p, :] = matmul lhsT.
    """
    NP = len(piece_meta)
    blocks = idx.reshape(B, NT, P)
    oh = np.zeros((B, P, NP, P), dtype=BF)
    for p, (k, ct) in enumerate(piece_meta):
        rel = blocks[:, k, :] - P * ct  # [B, 128]
        for b in range(B):
            tt = np.nonzero((rel[b] >= 0) & (rel[b] < P))[0]
            oh[b, rel[b, tt], p, tt] = 1
    return oh


# --------------------------------------------------------------- bass kernel

def _build_bass(piece_meta, has_b: bool, trivial_affine: bool, has_beta_row: bool):
    NP = len(piece_meta)
    # Bacc (not bare Bass): its compile() legalizes drains/sync-waits into
    # forms the walrus codegen accepts.
    nc = bacc.Bacc(None)

    xT = nc.declare_dram_parameter("xT", [D, C], BF16, isOutput=False)
    wt = nc.declare_dram_parameter("wt", [D, D], BF16, isOutput=False)
    oh = nc.declare_dram_parameter("oh", [P, NP, P], BF16, isOutput=False)
    if has_b:
        bvec = nc.declare_dram_parameter("bvec", [1, D], BF16, isOutput=False)
    if not trivial_affine:
        gamma = nc.declare_dram_parameter("gamma", [1, D], F32, isOutput=False)
        beta = nc.declare_dram_parameter("beta", [1, D], F32, isOutput=False)
    out = nc.declare_dram_parameter("out", [L, D], F32, isOutput=True)

    with tile.TileContext(nc) as tc:
        with (
            tc.tile_pool(name="singles", bufs=1) as singles,
            tc.tile_pool(name="stats", bufs=8) as stats_pool,
            tc.tile_pool(name="ostage_a", bufs=2) as ostage_a,
            tc.tile_pool(name="ostage_v", bufs=2) as ostage_v,
            tc.tile_pool(name="psA", bufs=2, space="PSUM") as psA,
            tc.tile_pool(name="psB", bufs=4, space="PSUM") as psB,
        ):
            # ---- resident loads (host provides layouts that DMA at line rate)
            xT_sb = singles.tile([P, KT, C], BF16)
            nc.sync.dma_start(
                out=xT_sb, in_=xT.rearrange("(kt p) c -> p kt c", p=P)
            )
            wt_sb = singles.tile([P, KT, D], BF16)
            nc.sync.dma_start(
                out=wt_sb, in_=wt.rearrange("(kt p) e -> p kt e", p=P)
            )
            oh_sb = singles.tile([P, NP, P], BF16)
            nc.sync.dma_start(out=oh_sb, in_=oh[:, :, :])

            eps_sb = singles.tile([P, 1], F32)
            nc.vector.memset(eps_sb, EPS)

            if has_b:
                ones_sb = singles.tile([1, P], BF16)
                nc.vector.memset(ones_sb, 1.0)
                bvec_sb = singles.tile([1, D], BF16)
                nc.sync.dma_start(out=bvec_sb, in_=bvec[:, :])

            if not trivial_affine:
                gamma_sb = singles.tile([P, D], F32)
                nc.gpsimd.dma_start(
                    out=gamma_sb, in_=gamma.to_broadcast((P, D))
                )
                beta_sb = singles.tile([P, D], F32)
                nc.gpsimd.dma_start(
                    out=beta_sb, in_=beta.to_broadcast((P, D))
                )

            # normalized projection table; tile CT holds the beta row (row 0)
            nproj = singles.tile([P, CT + 1, D], BF16)
            if has_beta_row:
                nc.vector.memset(nproj[:, CT, :], 0.0)
                if not trivial_affine:
                    nc.vector.tensor_copy(
                        out=nproj[0:1, CT, :], in_=beta_sb[0:1, :]
                    )

            # ---- stage A: proj = x @ W.T (+b), then LayerNorm -> nproj (bf16)
            for ct in range(CT):
                pj = psA.tile([P, D], F32)
                for kt in range(KT):
                    nc.tensor.matmul(
                        pj,
                        lhsT=xT_sb[:, kt, P * ct:P * (ct + 1)],
                        rhs=wt_sb[:, kt, :],
                        start=(kt == 0),
                        stop=(kt == KT - 1 and not has_b),
                    )
                if has_b:
                    nc.tensor.matmul(
                        pj, lhsT=ones_sb[:, :], rhs=bvec_sb[:, :],
                        start=False, stop=True,
                    )

                st = stats_pool.tile([P, 6], F32, tag="bn")
                nc.vector.bn_stats(out=st, in_=pj)
                mv = stats_pool.tile([P, 2], F32, tag="mv")
                nc.vector.bn_aggr(out=mv, in_=st)
                rstd = stats_pool.tile([P, 1], F32, tag="rstd")
                nc.scalar.activation(
                    out=rstd, in_=mv[:, 1:2],
                    func=mybir.ActivationFunctionType.Sqrt,
                    bias=eps_sb, scale=1.0,
                )
                nc.vector.reciprocal(out=rstd, in_=rstd)
                if trivial_affine:
                    nc.vector.tensor_scalar(
                        out=nproj[:, ct, :], in0=pj,
                        scalar1=mv[:, 0:1], scalar2=rstd,
                        op0=mybir.AluOpType.subtract, op1=mybir.AluOpType.mult,
                    )
                else:
                    tmp = stats_pool.tile([P, D], F32, tag="affine")
                    nc.vector.tensor_scalar(
                        out=tmp, in0=pj,
                        scalar1=mv[:, 0:1], scalar2=rstd,
                        op0=mybir.AluOpType.subtract, op1=mybir.AluOpType.mult,
                    )
                    nc.vector.tensor_mul(out=tmp, in0=tmp, in1=gamma_sb)
                    nc.vector.tensor_add(out=nproj[:, ct, :], in0=tmp, in1=beta_sb)

            # ---- stage B: ragged expansion as block-sparse one-hot matmuls
            by_block = [[] for _ in range(NT)]
            for p, (k, ct) in enumerate(piece_meta):
                by_block[k].append((p, ct))

            GRP = 4  # output blocks per store DMA (1 MiB batches)
            for g in range(NT // GRP):
                # separate pools per evacuation engine so copies never wait on
                # the other compute engine's semaphore (walrus has a low limit
                # on per-instruction sync waits)
                ot = (ostage_a if g % 2 == 0 else ostage_v).tile([P, GRP, D], F32)
                # tiny same-engine touch op claims the slot and absorbs the
                # DMA-release wait, keeping the big copies at <=2 sync waits
                # (walrus codegen rejects more)
                if g % 2 == 0:
                    nc.scalar.copy(out=ot[0:1, 0, 0:1], in_=eps_sb[0:1, 0:1])
                else:
                    nc.vector.memset(ot[0:1, 0, 0:1], 0.0)
                for ki in range(GRP):
                    k = g * GRP + ki
                    pieces = by_block[k]
                    go = psB.tile([P, D], F32)
                    for i, (p, ct) in enumerate(pieces):
                        nc.tensor.matmul(
                            go,
                            lhsT=oh_sb[:, p, :],
                            rhs=nproj[:, ct, :],
                            start=(i == 0),
                            stop=(i == len(pieces) - 1),
                        )
                    # alternate engines for PSUM evacuation (per store group, so
                    # the store DMA waits on a single engine's FIFO)
                    if g % 2 == 0:
                        nc.scalar.copy(out=ot[:, ki, :], in_=go)
                    else:
                        nc.vector.tensor_copy(out=ot[:, ki, :], in_=go)
                nc.sync.dma_start(
                    out=out[P * GRP * g:P * GRP * (g + 1), :].rearrange(
                        "(c p) e -> p c e", p=P
                    ),
                    in_=ot,
                )
    nc.compile()
    return nc


# ---------------------------------------------------------------- entrypoint

def kernel(**inputs) -> np.ndarray:
    return run_dechunk(inputs)[0]


def run_dechunk(inputs, trace=False, tmpdir=None):
    x = np.asarray(inputs["hierarchical_repr"], np.float32)
    W = np.asarray(inputs["W"], np.float32)
    bvec = np.asarray(inputs["b"], np.float32)
    gamma = np.asarray(inputs["gamma"], np.float32)
    beta = np.asarray(inputs["beta"], np.float32)
    cl = np.asarray(inputs["chunk_lengths"]).astype(np.int32)
    seq_len = int(np.asarray(inputs["seq_len"]))
    assert x.shape == (B, C, D) and W.shape == (D, D) and seq_len == L

    idx = _build_indices(cl)
    piece_meta = _build_pieces(idx)
    oh = _build_onehot(idx, piece_meta)

    has_b = bool(np.any(bvec != 0))
    trivial_affine = bool(np.all(gamma == 1.0) and np.all(beta == 0.0))
    has_beta_row = bool((idx == C).any())

    nc = _build_bass(piece_meta, has_b, trivial_affine, has_beta_row)

    wt_h = np.ascontiguousarray(W.T).astype(BF)
    in_maps = []
    for b in range(B):
        m = {
            "xT": np.ascontiguousarray(x[b].T).astype(BF),
            "wt": wt_h,
            "oh": oh[b],
        }
        if has_b:
            m["bvec"] = bvec.reshape(1, D).astype(BF)
        if not trivial_affine:
            m["gamma"] = gamma.reshape(1, D)
            m["beta"] = beta.reshape(1, D)
        in_maps.append(m)

    res = run_bass_kernel_spmd(
        nc, in_maps, core_ids=list(range(B)), trace=trace, tmpdir=tmpdir
    )
    out = np.stack(
        [np.asarray(res.results[i]["out"], np.float32) for i in range(B)]
    )
    return out, res
